# revision 1
# baseline (speedup 1.0000x reference)
"""Data-parallel Trainium kernel for the attention-LSTM decoder.

Shards batch B=512 across 8 NeuronCores (64 rows/core); all parameters are
replicated. The per-step recurrence is local to each core, so there is no
cross-device traffic. Executed via jax.pmap on the neuron PJRT backend.
"""
import numpy as np

B, T, INPUT, HID, NCLS, NSTEPS = 512, 64, 512, 512, 96, 27
NCORES = 8
BL = B // NCORES  # 64 rows per core


def _build():
    import jax
    import jax.numpy as jnp

    def local_forward(batch_H, text, W_i2h, W_h2h, b_h2h, W_score, W_ih, b_ih,
                      W_hh, b_hh, W_gen, b_gen):
        H = HID
        batch_H_proj = jnp.einsum("bti,hi->bth", batch_H, W_i2h)
        onehots = jnp.transpose(
            jax.nn.one_hot(text, NCLS, dtype=batch_H.dtype), (1, 0, 2))

        def step(carry, char_onehot):
            h, c = carry
            prev_proj = h @ W_h2h.T + b_h2h
            e = jnp.tanh(batch_H_proj + prev_proj[:, None, :]) @ W_score[0]
            alpha = jax.nn.softmax(e, axis=1)
            context = jnp.einsum("bt,bti->bi", alpha, batch_H)
            x = jnp.concatenate([context, char_onehot], axis=1)
            gates = x @ W_ih.T + b_ih + h @ W_hh.T + b_hh
            i_g = jax.nn.sigmoid(gates[:, 0 * H:1 * H])
            f_g = jax.nn.sigmoid(gates[:, 1 * H:2 * H])
            g_g = jnp.tanh(gates[:, 2 * H:3 * H])
            o_g = jax.nn.sigmoid(gates[:, 3 * H:4 * H])
            c_new = f_g * c + i_g * g_g
            h_new = o_g * jnp.tanh(c_new)
            return (h_new, c_new), h_new

        h0 = jnp.zeros((batch_H.shape[0], H), batch_H.dtype)
        c0 = jnp.zeros_like(h0)
        _, hiddens = jax.lax.scan(step, (h0, c0), onehots)
        output_hiddens = jnp.transpose(hiddens, (1, 0, 2))
        return jnp.einsum("bsh,ch->bsc", output_hiddens, W_gen) + b_gen

    return jax, local_forward


def kernel(**inputs) -> np.ndarray:
    jax, local_forward = _build()

    batch_H = np.asarray(inputs["batch_H"], dtype=np.float32)
    text = np.asarray(inputs["text"])
    params = [np.asarray(inputs[k], dtype=np.float32) for k in
              ("W_i2h", "W_h2h", "b_h2h", "W_score", "W_ih", "b_ih",
               "W_hh", "b_hh", "W_gen", "b_gen")]

    bh_sh = batch_H.reshape(NCORES, BL, T, INPUT)
    text_sh = text.reshape(NCORES, BL, NSTEPS).astype(np.int32)

    devs = jax.devices()
    n = min(NCORES, len(devs))
    fn = jax.pmap(local_forward,
                  in_axes=(0, 0) + (None,) * 10,
                  devices=devs[:n])
    out = fn(bh_sh, text_sh, *params)
    return np.asarray(out, dtype=np.float32).reshape(B, NSTEPS, NCLS)


if __name__ == "__main__":
    rng = np.random.default_rng(0)
    dummy = {
        "batch_H": rng.standard_normal((B, T, INPUT), dtype=np.float32),
        "text": rng.integers(0, NCLS, size=(B, NSTEPS)).astype(np.int64),
        "W_i2h": rng.standard_normal((HID, INPUT), dtype=np.float32) * 0.02,
        "W_h2h": rng.standard_normal((HID, HID), dtype=np.float32) * 0.02,
        "b_h2h": rng.standard_normal(HID, dtype=np.float32) * 0.02,
        "W_score": rng.standard_normal((1, HID), dtype=np.float32) * 0.02,
        "W_ih": rng.standard_normal((4 * HID, INPUT + NCLS), dtype=np.float32) * 0.02,
        "b_ih": rng.standard_normal(4 * HID, dtype=np.float32) * 0.02,
        "W_hh": rng.standard_normal((4 * HID, HID), dtype=np.float32) * 0.02,
        "b_hh": rng.standard_normal(4 * HID, dtype=np.float32) * 0.02,
        "W_gen": rng.standard_normal((NCLS, HID), dtype=np.float32) * 0.02,
        "b_gen": rng.standard_normal(NCLS, dtype=np.float32) * 0.02,
    }
    out = kernel(**dummy)
    print("out", out.shape, out.dtype)


# revision 2
# speedup vs baseline: 1.1914x; 1.1914x over previous
"""Data-parallel Trainium kernel for the attention-LSTM decoder.

Shards batch B=512 across 8 NeuronCores (64 rows/core); all parameters are
replicated. The per-step recurrence is local to each core, so there is no
cross-device traffic. Executed via jax.pmap on the neuron PJRT backend.
"""
import numpy as np

B, T, INPUT, HID, NCLS, NSTEPS = 512, 64, 512, 512, 96, 27
NCORES = 8
BL = B // NCORES  # 64 rows per core


def _build():
    import jax
    import jax.numpy as jnp

    def local_forward(batch_H, text, W_i2h, W_h2h, b_h2h, W_score, W_ih, b_ih,
                      W_hh, b_hh, W_gen, b_gen):
        H = HID
        batch_H_proj = jnp.einsum("bti,hi->bth", batch_H, W_i2h)
        onehots = jnp.transpose(
            jax.nn.one_hot(text, NCLS, dtype=batch_H.dtype), (1, 0, 2))

        def step(carry, char_onehot):
            h, c = carry
            prev_proj = h @ W_h2h.T + b_h2h
            e = jnp.tanh(batch_H_proj + prev_proj[:, None, :]) @ W_score[0]
            alpha = jax.nn.softmax(e, axis=1)
            context = jnp.einsum("bt,bti->bi", alpha, batch_H)
            x = jnp.concatenate([context, char_onehot], axis=1)
            gates = x @ W_ih.T + b_ih + h @ W_hh.T + b_hh
            i_g = jax.nn.sigmoid(gates[:, 0 * H:1 * H])
            f_g = jax.nn.sigmoid(gates[:, 1 * H:2 * H])
            g_g = jnp.tanh(gates[:, 2 * H:3 * H])
            o_g = jax.nn.sigmoid(gates[:, 3 * H:4 * H])
            c_new = f_g * c + i_g * g_g
            h_new = o_g * jnp.tanh(c_new)
            return (h_new, c_new), h_new

        h0 = jnp.zeros((batch_H.shape[0], H), batch_H.dtype)
        c0 = jnp.zeros_like(h0)
        _, hiddens = jax.lax.scan(step, (h0, c0), onehots)
        output_hiddens = jnp.transpose(hiddens, (1, 0, 2))
        return jnp.einsum("bsh,ch->bsc", output_hiddens, W_gen) + b_gen

    return jax, local_forward


_CACHE = {}


def kernel(**inputs) -> np.ndarray:
    if "fn" not in _CACHE:
        jax, local_forward = _build()
        devs = [d for d in jax.devices() if d.platform != "cpu"] or jax.devices()
        if len(devs) >= NCORES:
            fn = jax.pmap(local_forward, in_axes=(0, 0) + (None,) * 10,
                          devices=devs[:NCORES])
            _CACHE["fn"] = lambda bh, tx, *p: fn(
                bh.reshape(NCORES, BL, T, INPUT),
                tx.reshape(NCORES, BL, NSTEPS), *p)
        else:
            _CACHE["fn"] = jax.jit(local_forward)

    batch_H = np.asarray(inputs["batch_H"], dtype=np.float32)
    text = np.asarray(inputs["text"]).astype(np.int32)
    params = [np.asarray(inputs[k], dtype=np.float32) for k in
              ("W_i2h", "W_h2h", "b_h2h", "W_score", "W_ih", "b_ih",
               "W_hh", "b_hh", "W_gen", "b_gen")]

    out = _CACHE["fn"](batch_H, text, *params)
    return np.asarray(out, dtype=np.float32).reshape(B, NSTEPS, NCLS)


if __name__ == "__main__":
    rng = np.random.default_rng(0)
    dummy = {
        "batch_H": rng.standard_normal((B, T, INPUT), dtype=np.float32),
        "text": rng.integers(0, NCLS, size=(B, NSTEPS)).astype(np.int64),
        "W_i2h": rng.standard_normal((HID, INPUT), dtype=np.float32) * 0.02,
        "W_h2h": rng.standard_normal((HID, HID), dtype=np.float32) * 0.02,
        "b_h2h": rng.standard_normal(HID, dtype=np.float32) * 0.02,
        "W_score": rng.standard_normal((1, HID), dtype=np.float32) * 0.02,
        "W_ih": rng.standard_normal((4 * HID, INPUT + NCLS), dtype=np.float32) * 0.02,
        "b_ih": rng.standard_normal(4 * HID, dtype=np.float32) * 0.02,
        "W_hh": rng.standard_normal((4 * HID, HID), dtype=np.float32) * 0.02,
        "b_hh": rng.standard_normal(4 * HID, dtype=np.float32) * 0.02,
        "W_gen": rng.standard_normal((NCLS, HID), dtype=np.float32) * 0.02,
        "b_gen": rng.standard_normal(NCLS, dtype=np.float32) * 0.02,
    }
    out = kernel(**dummy)
    print("out", out.shape, out.dtype)
